# revision 18
# baseline (speedup 1.0000x reference)
"""CIN (Compressed Interaction Network) forward kernel for Trainium2.

Reference computation (per layer k, with x0 = inputs [B, F0, D]):
    x_k[b,h,d] = sum_{i,j} x_{k-1}[b,i,d] * x0[b,j,d] * Wr_k[i,j,h]
    pooled_k[b,h] = sum_d x_k[b,h,d]
    out = concat([pooled_0, pooled_1, pooled_2], axis=1)    # [B, 384]

Shapes: B=1024, F0=39, D=16, H=128; W0 [1521,128], W1/W2 [4992,128].

Strategy (pure data-parallel over B across 8 cores; Bc=128/core,
ntok = Bc*D = 2048 tokens, tok=(b,d) with d innermost):

  Per layer the contraction is out_T[h, t] = sum_K W[K,h] * z[K,t] with
  z[(j,i), t] = x_prev[i,t] * x0[j,t] (Khatri-Rao product, j-major blocks).
  - The partition-broadcast of x0 row j (V_rep) is built on the PE via a
    selector matmul (E_j^T @ x0T, stationary one-hot selector).
  - z_j = x_prev .* V_rep on the vector engine (one tensor_tensor per block).
  - Layer matmul: W_j block stationary [128,128], z_j moving [128,512],
    accumulating over j in PSUM.
  - L0 packs 3 j-blocks of 39 rows into K-tiles of 117 via a 3-hot selector.
  - L2 only needs pooled output, so sum over d first:
    pooled2[h,b] = sum_{i,j} W2r[i,j,h] * zsum[i,j,b],
    zsum[i,j,b] = sum_d x2[i,(b,d)] * x0[j,(b,d)]  -- tiny K=16 per-sample
    matmuls on PE-transposed x2, then a K=128 contraction accumulated over j.

Sync-wait legalization: TRN2 instructions carry at most 2 semaphore waits
(self-loading fp32 matmuls: 1). Tile doesn't know this, so the kernel
a) gives every input its own DRAM param (one DMA -> one queue sem),
b) issues tiny bf16 "absorber" matmuls that advance the PE's vector clock
   past DMA/ACT/Pool semaphores right before wait-heavy phases,
c) patches the kernel-tail drain to spill excess waits into standalone
   wait_ge instructions.
"""

import os
import numpy as np

try:
    import concourse.bass as bass  # noqa
except ImportError:  # pragma: no cover - fallback for odd sys.path setups
    import sys

    sys.path.insert(0, "/opt/trn_rl_repo")

import concourse.bass as bass
import concourse.mybir as mybir
import concourse.tile as tile_mod
from concourse.bass_utils import run_bass_kernel_spmd
from concourse.tile import TileContext
from concourse.tile_rust import add_dep_helper
from concourse.masks import make_identity
from contextlib import ExitStack

B, F0, D, H = 1024, 39, 16, 128
NCORES = 8
BC = B // NCORES          # 128 batch rows per core
NTOK = BC * D             # 2048 tokens per core
TT = 1024                 # token tile (2 tiles of 1024; PSUM accum = 2 banks)
NTT = NTOK // TT
KT0 = 13                  # L0 K-tiles of 117 = 3 j-blocks x 39
F32 = mybir.dt.float32
BF16 = mybir.dt.bfloat16
F16 = mybir.dt.float16

# "f32" (exact, 4 cyc/row) or "f16" (1 cyc/row, ~1e-3 rel err)
MM_DT = os.environ.get("CIN_MM_DT", "f32")
MDT = {"f32": F32, "f16": F16}[MM_DT]
NPDT = {"f32": np.float32, "f16": np.float16}[MM_DT]

_MAX_DRAIN_WAITS = 1


def _legalize_waits(wait_clock):
    """Walrus on this toolchain accepts a single sync-wait per instruction.
    Keep the first wait and spill the rest into standalone InstEventSemaphore
    (wait_ge) instructions inserted just before, on the same engine."""
    import bass_rust as _br

    by_name = {h.name: h for h in wait_clock.sems.allocated().values()}
    ctr = 0
    for insts in wait_clock.ordered_instructions_by_block.values():
        i = 0
        while i < len(insts):
            inst = insts[i]
            si = getattr(inst, "sync_info", None)
            eng = getattr(inst, "engine", None)
            ow = list(si.on_wait) if si is not None and si.on_wait else []
            if (
                len(ow) > 1
                and eng is not None
                and eng != mybir.EngineType.Unassigned
            ):
                si.on_wait = ow[:1]
                new = []
                for w in ow[1:]:
                    h = by_name.get(w.ant_name)
                    if h is None:
                        # unknown sem (shouldn't happen) -- keep it in place
                        si.on_wait = list(si.on_wait) + [w]
                        continue
                    ev = mybir.InstEventSemaphore(
                        name=f"I-wspill-{id(wait_clock)}-{ctr}", ins=[], outs=[]
                    )
                    ctr += 1
                    ev.engine = eng
                    _br.wait_op(ev, h, w.wait_value, "sem-ge", True)
                    new.append(ev)
                insts[i:i] = new
                i += len(new)
            i += 1


def _patch_clock_wait():
    if getattr(tile_mod, "_cin_clockwait_patched", False):
        return
    Orig = tile_mod.TileClockWait

    class LegalizingClockWait:
        """Delegating wrapper (rust class is final): legalize after the
        top-level assign_waits call."""

        def __init__(self, *a, **k):
            self._inner = Orig(*a, **k)

        def assign_waits(self, bb_name):
            self._inner.assign_waits(bb_name)
            _legalize_waits(self._inner)

        def __getattr__(self, n):
            return getattr(self._inner, n)

    tile_mod.TileClockWait = LegalizingClockWait
    tile_mod._cin_clockwait_patched = True


def _patch_drain():
    """Spill kernel-tail drain waits beyond 2 into standalone wait_ge's."""
    if getattr(tile_mod.TileContext._drain_and_barrier, "_cin_patched", False):
        return

    def _drain_and_barrier(self, tick_clock, wait_clock):
        from concourse.vector_clock import ScopedClock

        nc = self.nc
        drain_inst = nc.sync.drain()
        wait_clock.add_sem_waits(
            drain_inst.ins, ScopedClock({None: tick_clock.global_clock})
        )
        si = drain_inst.ins.sync_info
        waits = list(si.on_wait) if si and si.on_wait else []
        if len(waits) > _MAX_DRAIN_WAITS:
            keep, spill = waits[:_MAX_DRAIN_WAITS], waits[_MAX_DRAIN_WAITS:]
            si.on_wait = keep
            assert self.sems is not None
            by_name = {h.name: h for h in self.sems.allocated().values()}
            for w in spill:
                h = by_name[w.ant_name]
                nc.sync.wait_ge(h, w.wait_value)

        nc.all_engine_barrier()
        popped = nc._tile_sem_poison_stack.pop()
        assert popped is self._sem_poison
        nc.clear_and_free_semaphores(list(self.sems.allocated().values()))
        nc.all_engine_barrier()

    _drain_and_barrier._cin_patched = True
    tile_mod.TileContext._drain_and_barrier = _drain_and_barrier


def _build_bass():
    _patch_drain()
    _patch_clock_wait()
    nc = bass.Bass()

    x0T_d = nc.declare_dram_parameter("x0T", [F0, NTOK], MDT, isOutput=False)
    x0r3_d = nc.declare_dram_parameter("x0r3", [117, NTOK], F32, isOutput=False)
    x0D_d = nc.declare_dram_parameter("x0D", [128, BC * F0], MDT, isOutput=False)
    W0p_d = nc.declare_dram_parameter("W0p", [117, KT0 * H], MDT, isOutput=False)
    W1p_d = nc.declare_dram_parameter("W1p", [H, F0 * H], MDT, isOutput=False)
    W2p_d = nc.declare_dram_parameter("W2p", [H, F0 * H], MDT, isOutput=False)
    sel0_d = nc.declare_dram_parameter("sel0", [F0, KT0 * 117], MDT, isOutput=False)
    sel39_d = nc.declare_dram_parameter("sel39", [F0, F0 * H], MDT, isOutput=False)
    out_d = nc.declare_dram_parameter("out", [3 * H, BC], F32, isOutput=True)

    with TileContext(nc) as tc, ExitStack() as ctx:
        const = ctx.enter_context(tc.tile_pool(name="const", bufs=1))
        zpool = ctx.enter_context(tc.tile_pool(name="z", bufs=3))
        vrep = ctx.enter_context(tc.tile_pool(name="vrep", bufs=2, space="PSUM"))
        accum = ctx.enter_context(tc.tile_pool(name="accum", bufs=2, space="PSUM"))

        # ---- constants / inputs in SBUF (one DMA each) ----
        x0T = const.tile([F0, NTOK], MDT)
        nc.sync.dma_start(x0T[:], x0T_d[:])
        x0r3 = const.tile([117, NTOK], F32)
        nc.sync.dma_start(x0r3[:], x0r3_d[:])
        x0D = const.tile([128, BC * F0], MDT)
        nc.sync.dma_start(x0D[:], x0D_d[:])
        W0p = const.tile([117, KT0 * H], MDT)
        nc.sync.dma_start(W0p[:], W0p_d[:])
        W1p = const.tile([H, F0 * H], MDT)
        nc.sync.dma_start(W1p[:], W1p_d[:])
        W2p = const.tile([H, F0 * H], MDT)
        nc.sync.dma_start(W2p[:], W2p_d[:])
        sel0 = const.tile([F0, KT0 * 117], MDT)
        nc.sync.dma_start(sel0[:], sel0_d[:])
        sel39 = const.tile([F0, F0 * H], MDT)
        nc.sync.dma_start(sel39[:], sel39_d[:])
        ident = const.tile([128, 128], F32)
        make_identity(nc, ident[:])

        x1_sb = const.tile([H, NTOK], F32)
        x2_sb = const.tile([H, 2 * NTOK], F32)  # col = b*32 + d, zero-padded
        x2t_sb = const.tile([128, 32 * H], MDT)
        zsum_sb = const.tile([H, BC * F0], MDT)
        pooled0 = const.tile([H, BC], F32)
        pooled1 = const.tile([H, BC], F32)
        pooled2 = const.tile([H, BC], F32)

        mult = mybir.AluOpType.mult
        nc.gpsimd.memset(x2_sb[:], 0.0)

        # PE absorber: standalone bf16 ldweights reading a couple elements of
        # each tile; advances the PE vector clock past the producers' sems so
        # the next matmul (1 wait slot on this walrus) carries at most one
        # fresh wait. The garbage weight load is harmless: every real matmul
        # loads its own weights.
        def pe_absorb(*aps):
            return [nc.tensor.ldweights(ap.bitcast(BF16)) for ap in aps]

        dve_dummy = const.tile([1, 8], F32)

        # absorb all input-load DMA queue sems on the PE up front
        pe_absorb(x0T[0:1, 0:2], W0p[0:1, 0:2])
        pe_absorb(W1p[0:1, 0:2], W2p[0:1, 0:2])
        pe_absorb(sel0[0:1, 0:2], sel39[0:1, 0:2])
        pe_absorb(x0D[0:1, 0:2], x0r3[0:1, 0:2])

        # ================= Layer 0 =================
        # K = 1521 (j-major), tiled as 13 x 117 (3 j-blocks of 39 rows).
        for tt in range(NTT):
            sl = slice(tt * TT, (tt + 1) * TT)
            x1_ps = accum.tile([128, TT], F32, tag="accum")
            for kt in range(KT0):
                v0 = vrep.tile([128, TT], F32, tag="vrep")
                for hh in range(TT // 512):
                    hs = slice(tt * TT + hh * 512, tt * TT + (hh + 1) * 512)
                    nc.tensor.matmul(
                        v0[:117, hh * 512 : (hh + 1) * 512],
                        sel0[:, kt * 117 : (kt + 1) * 117],
                        x0T[:, hs],
                        tile_position=(0, 0),
                        skip_group_check=True,
                    )
                z = zpool.tile([128, TT], MDT, tag="z")
                nc.vector.tensor_tensor(z[:117, :], x0r3[:, sl], v0[:117, :], mult)
                for hh in range(TT // 512):
                    nc.tensor.matmul(
                        x1_ps[:, hh * 512 : (hh + 1) * 512],
                        W0p[:, kt * H : (kt + 1) * H],
                        z[:117, hh * 512 : (hh + 1) * 512],
                        start=(kt == 0),
                        stop=(kt == KT0 - 1),
                        tile_position=(0, 0),
                        skip_group_check=True,
                    )
            nc.scalar.copy(x1_sb[:, sl], x1_ps[:, :])

        nc.vector.tensor_reduce(
            pooled0[:, :],
            x1_sb[:].rearrange("p (b d) -> p b d", d=D),
            mybir.AxisListType.X,
            mybir.AluOpType.add,
        )
        nc.sync.dma_start(out_d[0:H, :], pooled0[:, :])

        # ================= Layer 1 =================
        # K = 39*128 (j-major blocks of 128); V_rep via one-hot selector.
        for tt in range(NTT):
            sl = slice(tt * TT, (tt + 1) * TT)
            # absorb the ACT copy (x1 / psum-slot release) before the fp32 mms
            dd = nc.vector.tensor_copy(
                dve_dummy[0:1, 0:2], x1_sb[0:1, sl.stop - 2 : sl.stop]
            )
            ab = pe_absorb(x1_sb[0:1, sl.stop - 2 : sl.stop])
            x2_ps = accum.tile([128, TT], F32, tag="accum")
            first_tt = first_mm = None
            for j in range(F0):
                v = vrep.tile([128, TT], F32, tag="vrep")
                for hh in range(TT // 512):
                    hs = slice(tt * TT + hh * 512, tt * TT + (hh + 1) * 512)
                    nc.tensor.matmul(
                        v[:, hh * 512 : (hh + 1) * 512],
                        sel39[:, j * H : (j + 1) * H],
                        x0T[:, hs],
                        tile_position=(0, 0),
                        skip_group_check=True,
                    )
                z = zpool.tile([128, TT], MDT, tag="z")
                ti = nc.vector.tensor_tensor(z[:, :], x1_sb[:, sl], v[:, :], mult)
                if first_tt is None:
                    first_tt = ti
                    add_dep_helper(ti.ins, dd.ins, sync=True, reason="absorb")
                for hh in range(TT // 512):
                    mi = nc.tensor.matmul(
                        x2_ps[:, hh * 512 : (hh + 1) * 512],
                        W1p[:, j * H : (j + 1) * H],
                        z[:, hh * 512 : (hh + 1) * 512],
                        start=(j == 0),
                        stop=(j == F0 - 1),
                        tile_position=(0, 0),
                        skip_group_check=True,
                    )
                    if first_mm is None:
                        first_mm = mi
                        for a in ab:
                            add_dep_helper(mi.ins, a.ins, sync=True, reason="absorb")
            x2_dst = x2_sb[:].rearrange("p (b g) -> p b g", g=32)[
                :, tt * (TT // D) : (tt + 1) * (TT // D), 0:D
            ]
            nc.scalar.copy(x2_dst, x2_ps[:].rearrange("p (b d) -> p b d", d=D))

        nc.vector.tensor_reduce(
            pooled1[:, :],
            x2_sb[:].rearrange("p (b g) -> p b g", g=32)[:, :, 0:D],
            mybir.AxisListType.X,
            mybir.AluOpType.add,
        )
        nc.sync.dma_start(out_d[H : 2 * H, :], pooled1[:, :])

        # ================= Layer 2 (pooled only, via zsum) =================
        # Step 1: PE-transpose x2 into d-major layout (zero-interleaved so
        # every sample's 16 d-rows land at a 32-aligned partition base).
        ab = pe_absorb(ident[0:1, 0:2], x2_sb[0:1, 127 * 32 + 14 : 127 * 32 + 16])
        for g in range(32):  # 4 samples per transpose
            x2t_ps = vrep.tile([128, 128], F32, tag="vrep")
            tri = nc.tensor.transpose(
                x2t_ps[:, :],
                x2_sb[:, g * 128 : (g + 1) * 128],
                ident[:, :],
            )
            if g == 0:
                for a in ab:
                    add_dep_helper(tri.ins, a.ins, sync=True, reason="absorb")
            nc.scalar.copy(x2t_sb[:, g * H : (g + 1) * H], x2t_ps[:, :])

        # Step 2: zsum via K=128 matmuls per 4-sample group; the rhs x0D is
        # zero-padded so each sample's columns only engage its own 16 d-rows
        # (row-group tile_position on fp32 matmuls faults at runtime here).
        # zsum_sb[i, b*39+j] = sum_d x2[i,(b,d)] * x0[j,(b,d)]
        for G in range(16):  # 8 samples (2 groups) per psum tile: [128, 312]
            # absorb both the x2t copies this group reads AND the ACT copy
            # that released this accum slot (zsum copy G-2) -- ACT sem is
            # monotone so reading the later-tick region covers both.
            slot_rel = (
                zsum_sb[0:1, (G - 2) * 8 * F0 : (G - 2) * 8 * F0 + 2]
                if G >= 2
                else x2_sb[0:1, 127 * 32 + 14 : 127 * 32 + 16]
            )
            ab = pe_absorb(
                x2t_sb[0:1, (2 * G + 1) * H : (2 * G + 1) * H + 2], slot_rel
            )
            zs_ps = accum.tile([128, 8 * F0], F32, tag="accum")
            for gg in range(2):
                g = 2 * G + gg
                mi = nc.tensor.matmul(
                    zs_ps[:, gg * 4 * F0 : (gg + 1) * 4 * F0],
                    x2t_sb[:, g * H : (g + 1) * H],
                    x0D[:, 4 * g * F0 : 4 * (g + 1) * F0],
                    tile_position=(0, 0),
                    skip_group_check=True,
                )
                if gg == 0:
                    for a in ab:
                        add_dep_helper(mi.ins, a.ins, sync=True, reason="absorb")
            nc.scalar.copy(
                zsum_sb[:, G * 8 * F0 : (G + 1) * 8 * F0], zs_ps[:, :]
            )

        # Step 3: pooled2[h,b] = sum_j W2p_j^T zsum_j (zsum cols strided by 39)
        ab = pe_absorb(
            zsum_sb[0:1, 15 * 8 * F0 : 15 * 8 * F0 + 2],
            zsum_sb[0:1, 14 * 8 * F0 : 14 * 8 * F0 + 2],
        )
        zsum_r = zsum_sb[:].rearrange("p (b j) -> p b j", j=F0)
        p2_ps = vrep.tile([128, BC], F32, tag="vrep")
        for j in range(F0):
            mi = nc.tensor.matmul(
                p2_ps[:, :],
                W2p[:, j * H : (j + 1) * H],
                zsum_r[:, :, j],
                start=(j == 0),
                stop=(j == F0 - 1),
                tile_position=(0, 0),
                skip_group_check=True,
            )
            if j == 0:
                for a in ab:
                    add_dep_helper(mi.ins, a.ins, sync=True, reason="absorb")
        nc.scalar.copy(pooled2[:, :], p2_ps[:, :])
        nc.sync.dma_start(out_d[2 * H : 3 * H, :], pooled2[:, :])

    return nc


_CACHE = {}


def _get_bass():
    key = ("nc", MM_DT)
    if key not in _CACHE:
        _CACHE[key] = _build_bass()
    return _CACHE[key]


def _prep_shared(W0, W1, W2):
    # j-major permutation of each layer's weights.
    W0r = W0.reshape(F0, F0, H)                      # [i, j, h]
    W0p = np.ascontiguousarray(W0r.transpose(1, 0, 2)).reshape(F0 * F0, H)
    W0p_sb = np.ascontiguousarray(
        W0p.reshape(KT0, 117, H).transpose(1, 0, 2)
    ).reshape(117, KT0 * H).astype(NPDT)
    W1p_sb = (
        np.ascontiguousarray(W1.reshape(H, F0, H)).reshape(H, F0 * H).astype(NPDT)
    )
    W2p_sb = (
        np.ascontiguousarray(W2.reshape(H, F0, H)).reshape(H, F0 * H).astype(NPDT)
    )
    sel0 = np.zeros((F0, KT0 * 117), NPDT)
    for kt in range(KT0):
        for c in range(3):
            sel0[3 * kt + c, kt * 117 + c * 39 : kt * 117 + (c + 1) * 39] = 1.0
    sel39 = np.zeros((F0, F0 * H), NPDT)
    for j in range(F0):
        sel39[j, j * H : (j + 1) * H] = 1.0
    return W0p_sb, W1p_sb, W2p_sb, sel0, sel39


def _prep_core(x0c):
    # x0c [BC, F0, D] -> x0T [F0, NTOK] (tok = b*D + d)
    x0T = np.ascontiguousarray(x0c.transpose(1, 0, 2)).reshape(F0, NTOK)
    x0r3 = np.concatenate([x0T, x0T, x0T], axis=0)   # [117, NTOK]
    # x0D[32q+d, b*39+j] = x0c[b, j, d] if b % 4 == q else 0
    x0D = np.zeros((4, 32, BC, F0), np.float32)
    bidx = np.arange(BC)
    xp = x0c.transpose(2, 0, 1)                     # [d, b, j]
    for q in range(4):
        m = bidx % 4 == q
        x0D[q, :D, m, :] = xp[:, m, :].transpose(1, 0, 2)
    x0D = x0D.reshape(128, BC * F0)
    return x0T.astype(NPDT), x0r3.astype(np.float32), x0D.astype(NPDT)


def kernel(inputs, W0, W1, W2):
    inputs = np.asarray(inputs, np.float32)
    W0 = np.asarray(W0, np.float32)
    W1 = np.asarray(W1, np.float32)
    W2 = np.asarray(W2, np.float32)

    W0p_sb, W1p_sb, W2p_sb, sel0, sel39 = _prep_shared(W0, W1, W2)
    in_maps = []
    for c in range(NCORES):
        x0T, x0r3, x0D = _prep_core(inputs[c * BC : (c + 1) * BC])
        in_maps.append(
            {
                "x0T": x0T,
                "x0r3": x0r3,
                "x0D": x0D,
                "W0p": W0p_sb,
                "W1p": W1p_sb,
                "W2p": W2p_sb,
                "sel0": sel0,
                "sel39": sel39,
            }
        )

    nc = _get_bass()
    res = run_bass_kernel_spmd(nc, in_maps, list(range(NCORES)))
    out = np.empty((B, 3 * H), np.float32)
    for c in range(NCORES):
        out[c * BC : (c + 1) * BC, :] = res.results[c]["out"].T
    return out
